# revision 1
# baseline (speedup 1.0000x reference)
"""Causal multi-head attention (B=128, T=256, C=384, H=6, Dh=64) on 8 TRN2
NeuronCores, data-parallel over batch (16 batches per core, no collectives).

Layout strategy per core:
  - host pre-transposes x to xT [b, C, T] and casts activations/weights to bf16
  - QT/KT computed as [D, T] (Dh on partitions) so scores = QT_h.T @ KT_h needs
    no on-chip transpose of Q/K
  - V computed as [T, D] so AV contraction (over key positions) has keys on
    partitions
  - softmax over the free dim (keys) without max-subtraction (scores are
    O(10) here, exp cannot overflow in fp32); row sums fused into the exp
    activation via accum_out
  - P is transposed on the PE (bf16, 1 cycle/row) for the AV matmul
  - output projection consumes OT [D, T] as the stationary operand directly
"""

import sys

sys.path.insert(0, "/opt/trn_rl_repo")

import numpy as np
import ml_dtypes

import concourse.bass as bass
import concourse.tile as tile
from concourse import mybir
from concourse.bass_utils import run_bass_kernel_spmd
from concourse.masks import make_causal_mask, make_identity

def split_multi_waits(nc):
    """This walrus build accepts at most one sync-wait command per
    instruction; hoist extra waits into standalone InstEventSemaphore
    instructions on the same engine queue (queue waits run in order before
    the original instruction, so semantics are preserved)."""
    ctr = [0]

    def mk(engine, wait):
        ctr[0] += 1
        return mybir.InstEventSemaphore(
            name=f"WSPLIT-{ctr[0]}",
            engine=engine,
            ins=[],
            outs=[],
            sync_info=mybir.SyncInfo(on_wait=[wait], on_update=[]),
        )

    for f in nc.m.functions:
        for blk in f.blocks:
            insts = blk.instructions
            out = []
            for inst in insts:
                si = inst.sync_info
                if si is not None and len(si.on_wait) > 1:
                    waits = list(si.on_wait)
                    for w in waits[:-1]:
                        out.append(mk(inst.engine, w))
                    inst.sync_info = mybir.SyncInfo(
                        on_wait=[waits[-1]], on_update=list(si.on_update)
                    )
                out.append(inst)
            insts[:] = out
    return nc


N_CORES = 8
B, T, C = 128, 256, 384
H, DH = 6, 64
BL = B // N_CORES  # batches per core
BF16 = mybir.dt.bfloat16
FP32 = mybir.dt.float32
AFT = mybir.ActivationFunctionType
SCALE = DH**-0.5  # 0.125
NEG = -1.0e9


def build_kernel() -> bass.Bass:
    nc = bass.Bass()
    xT = nc.dram_tensor("xT", [BL, C, T], BF16, kind="ExternalInput")
    wqt = nc.dram_tensor("wqt", [C, C], BF16, kind="ExternalInput")  # Wq.T [C, D]
    wkt = nc.dram_tensor("wkt", [C, C], BF16, kind="ExternalInput")
    wvt = nc.dram_tensor("wvt", [C, C], BF16, kind="ExternalInput")
    wot = nc.dram_tensor("wot", [C, C], BF16, kind="ExternalInput")  # Wo.T [D, C]
    y = nc.dram_tensor("y", [BL, T, C], FP32, kind="ExternalOutput")

    GB = 2  # batches per projection group (N = GB*T = 512 <= one PSUM bank fp32)
    with tile.TileContext(nc) as tc:
        with (
            tc.tile_pool(name="const", bufs=1) as const,
            tc.tile_pool(name="xp", bufs=2) as xp,
            tc.tile_pool(name="qkv", bufs=2) as qkv,
            tc.tile_pool(name="pp", bufs=3) as pp,
            tc.tile_pool(name="ptp", bufs=3) as ptp,
            tc.tile_pool(name="st", bufs=4) as st,
            tc.tile_pool(name="otp", bufs=2) as otp,
            tc.tile_pool(name="yp", bufs=3) as yp,
            tc.tile_pool(name="psA", bufs=6, space="PSUM") as psA,
            tc.tile_pool(name="psO", bufs=2, space="PSUM") as psO,
        ):
            ident = const.tile([128, 128], BF16)
            make_identity(nc, ident)
            # multiplicative 0/1 causal masks (bf16), applied post-exp
            m0 = const.tile([128, 128], BF16)
            nc.gpsimd.memset(m0, 1.0)
            nc.gpsimd.affine_select(
                out=m0, in_=m0, compare_op=mybir.AluOpType.is_ge,
                fill=0.0, base=0, pattern=[[-1, 128]], channel_multiplier=1,
            )
            # combined per-head mask over [tq0 keys 0:128 | tq1 keys 0:256]
            mc = const.tile([128, 384], BF16)
            nc.gpsimd.memset(mc, 1.0)
            nc.vector.tensor_copy(mc[:, 0:128], m0)
            nc.vector.tensor_copy(mc[:, 256:384], m0)

            w_sb = {}
            for name, dram in (("wq", wqt), ("wk", wkt), ("wv", wvt), ("wo", wot)):
                w = const.tile([128, 3, C], BF16, tag=name)
                nc.sync.dma_start(out=w, in_=dram.rearrange("(k p) d -> p k d", p=128))
                w_sb[name] = w

            for g in range(BL // GB):
                # ---- load xT for GB batches: [128, k, b, T] ----
                xt = xp.tile([128, 3, GB, T], BF16)
                for bi in range(GB):
                    nc.sync.dma_start(
                        out=xt[:, :, bi, :],
                        in_=xT[g * GB + bi].rearrange("(k p) t -> p k t", p=128),
                    )

                # ---- QT/KT for both batches: [D, b, T], N = GB*T ----
                qt = qkv.tile([128, 3, GB, T], BF16, tag="qt")
                kt = qkv.tile([128, 3, GB, T], BF16, tag="kt")
                for dst, wname in ((qt, "wq"), (kt, "wk")):
                    w = w_sb[wname]
                    for d in range(3):
                        ps = psA.tile([128, GB * T], FP32, tag="big")
                        for k in range(3):
                            nc.tensor.matmul(
                                ps,
                                lhsT=w[:, k, d * 128 : (d + 1) * 128],
                                rhs=xt[:, k, :, :],
                                start=(k == 0),
                                stop=(k == 2),
                            )
                        nc.any.tensor_copy(dst[:, d, :, :], ps)

                # ---- V = [T, D] per batch ----
                vs = []
                for bi in range(GB):
                    v = qkv.tile([128, 2, C], BF16, tag=f"v{bi}")
                    for t2 in range(2):
                        ps = psA.tile([128, GB * T], FP32, tag="big")
                        for k in range(3):
                            nc.tensor.matmul(
                                ps[:, 0:C],
                                lhsT=xt[:, k, bi, t2 * 128 : (t2 + 1) * 128],
                                rhs=w_sb["wv"][:, k, :],
                                start=(k == 0),
                                stop=(k == 2),
                            )
                        nc.any.tensor_copy(v[:, t2, :], ps[:, 0:C])
                    vs.append(v)

                for bi in range(GB):
                    b = g * GB + bi
                    v = vs[bi]
                    # ---- attention, one head-pair at a time ----
                    ot = otp.tile([128, 3, T], BF16)  # OT [D, T]
                    for pair in range(3):
                        po = psO.tile([128, T], FP32)
                        # scores: row-packed pair (even head rows 0:64, odd
                        # 64:128); per head one psum tile [128, 384] holding
                        # [tq0 x ts0:128 | tq1 x ts0:256]
                        sc = [None, None]
                        for sub in range(2):
                            sc[sub] = psA.tile(
                                [128, 384], FP32, tag="big", name=f"sc{sub}"
                            )
                        for blkid in range(2):
                            for sub in range(2):
                                doff = sub * 64
                                qh = qt[doff : doff + 64, pair, bi, :]
                                kh = kt[doff : doff + 64, pair, bi, :]
                                if blkid == 0:
                                    nc.tensor.matmul(
                                        sc[sub][:, 0:128],
                                        lhsT=qh[:, 0:128], rhs=kh[:, 0:128],
                                        start=True, stop=True,
                                    )
                                else:
                                    nc.tensor.matmul(
                                        sc[sub][:, 128:384],
                                        lhsT=qh[:, 128:T], rhs=kh,
                                        start=True, stop=True,
                                    )
                        # softmax: exp straight from PSUM (no max-subtraction),
                        # multiplicative causal mask, then row-normalize
                        ps_ = []
                        for sub in range(2):
                            p = pp.tile([128, 384], BF16, tag=f"p{sub}")
                            sums = st.tile([128, 2], FP32, tag=f"sums{sub}")
                            rs = st.tile([128, 2], FP32, tag=f"rs{sub}")
                            nc.scalar.activation(p, sc[sub], AFT.Exp, scale=SCALE)
                            nc.vector.tensor_mul(p, p, mc)
                            # partition p holds two queries: tq0 row p in cols
                            # 0:128 and tq1 row p in cols 128:384 — separate sums
                            nc.vector.reduce_sum(
                                out=sums[:, 0:1], in_=p[:, 0:128],
                                axis=mybir.AxisListType.X,
                            )
                            nc.vector.reduce_sum(
                                out=sums[:, 1:2], in_=p[:, 128:384],
                                axis=mybir.AxisListType.X,
                            )
                            nc.vector.reciprocal(rs, sums)
                            nc.vector.tensor_scalar_mul(
                                p[:, 0:128], p[:, 0:128], rs[:, 0:1]
                            )
                            nc.vector.tensor_scalar_mul(
                                p[:, 128:384], p[:, 128:384], rs[:, 1:2]
                            )
                            ps_.append(p)
                        # transpose P blocks on the PE: PT[ts, tq]
                        pts = []
                        for sub in range(2):
                            p = ps_[sub]
                            pt = ptp.tile([128, 2, T], BF16, tag=f"pt{sub}")
                            tp = psA.tile([128, T], BF16, tag="big", name="tp")
                            nc.tensor.transpose(tp[:, 0:128], p[:, 0:128], ident)
                            nc.tensor.transpose(tp[:, 128:T], p[:, 128:256], ident)
                            nc.any.tensor_copy(pt[:, 0, :], tp)
                            tp2 = psA.tile([128, 128], BF16, tag="big", name="tp2")
                            nc.tensor.transpose(tp2, p[:, 256:384], ident)
                            nc.any.tensor_copy(pt[:, 1, 128:T], tp2)
                            pts.append(pt)
                        # AV: col-packed pair; interleave even/odd for overlap
                        for mm in range(3):
                            for sub in range(2):
                                h = 2 * pair + sub
                                doff = sub * 64
                                pt = pts[sub]
                                out_ap = po[doff : doff + 64, :]
                                if mm == 0:
                                    nc.tensor.matmul(
                                        out_ap[:, 0:128],
                                        lhsT=v[:, 0, h * 64 : (h + 1) * 64],
                                        rhs=pt[:, 0, 0:128],
                                        start=True, stop=True,
                                        tile_position=(0, doff),
                                    )
                                else:
                                    ts_ = mm - 1
                                    nc.tensor.matmul(
                                        out_ap[:, 128:T],
                                        lhsT=v[:, ts_, h * 64 : (h + 1) * 64],
                                        rhs=pt[:, ts_, 128:T],
                                        start=(ts_ == 0), stop=(ts_ == 1),
                                        tile_position=(0, doff),
                                    )
                        nc.any.tensor_copy(ot[:, pair, :], po)

                    # ---- y = OT.T @ WoT : [T, C] ----
                    for t2 in range(2):
                        ps = psA.tile([128, GB * T], FP32, tag="big")
                        for k in range(3):
                            nc.tensor.matmul(
                                ps[:, 0:C],
                                lhsT=ot[:, k, t2 * 128 : (t2 + 1) * 128],
                                rhs=w_sb["wo"][:, k, :],
                                start=(k == 0),
                                stop=(k == 2),
                            )
                        ys = yp.tile([128, C], FP32)
                        nc.any.tensor_copy(ys, ps[:, 0:C])
                        nc.sync.dma_start(
                            out=y[b, t2 * 128 : (t2 + 1) * 128, :], in_=ys
                        )
    return nc


_NC = None


def _get_nc():
    global _NC
    if _NC is None:
        _NC = split_multi_waits(build_kernel())
    return _NC


def kernel(x, Wq, Wk, Wv, Wo, _trace=False):
    bf16 = ml_dtypes.bfloat16
    wq_t = np.ascontiguousarray(Wq.T).astype(bf16)
    wk_t = np.ascontiguousarray(Wk.T).astype(bf16)
    wv_t = np.ascontiguousarray(Wv.T).astype(bf16)
    wo_t = np.ascontiguousarray(Wo.T).astype(bf16)
    in_maps = []
    for i in range(N_CORES):
        xs = x[i * BL : (i + 1) * BL]  # [BL, T, C]
        xs_t = np.ascontiguousarray(xs.transpose(0, 2, 1)).astype(bf16)
        in_maps.append(
            {"xT": xs_t, "wqt": wq_t, "wkt": wk_t, "wvt": wv_t, "wot": wo_t}
        )
    res = run_bass_kernel_spmd(
        _get_nc(), in_maps, list(range(N_CORES)), trace=_trace
    )
    out = np.concatenate([r["y"] for r in res.results], axis=0)
    if _trace:
        return out.astype(np.float32), res
    return out.astype(np.float32)



# revision 3
# speedup vs baseline: 1.5983x; 1.5983x over previous
"""Causal multi-head attention (B=128, T=256, C=384, H=6, Dh=64) on 8 TRN2
NeuronCores, data-parallel over batch (16 batches per core, no collectives).

v2: transposed-scores formulation — P is never transposed.
  - scores computed directly as ST[ts, tq] = KT_h.T @ QT_h (K stationary), so
    exp(ST) = unnormalized P^T feeds the AV matmul with no transpose
  - causal mask applied by accumulating -1e9 * TRI into the score PSUM via a
    (-1e9*I) stationary matmul before the exp (exp -> exact 0)
  - AV uses the P^T blocks as the *stationary* operand and [V_h | ones] as
    moving, so O lands as [tq, d] with the softmax denominator in column 64
    -> normalization is a per-partition tensor_scalar (no cross-partition
    broadcast needed)
  - O is transposed back to OT [d, tq] on the PE (6 bf16 128x128 transposes
    per batch, ~30x less transpose work than transposing P)
  - output projection consumes OT [D, tq] as stationary; PSUM pressure: one
    shared "big" pool (QK/V/O) + scores + OT + Y = 8 banks exactly
"""

import sys

sys.path.insert(0, "/opt/trn_rl_repo")

import numpy as np
import ml_dtypes

import concourse.bass as bass
import concourse.tile as tile
from concourse import mybir
from concourse.bass_utils import run_bass_kernel_spmd
from concourse.masks import make_identity


def split_multi_waits(nc):
    """This walrus build accepts at most one sync-wait command per
    instruction; hoist extra waits into standalone InstEventSemaphore
    instructions on the same engine queue (queue waits run in order before
    the original instruction, so semantics are preserved)."""
    ctr = [0]

    def mk(engine, wait):
        ctr[0] += 1
        return mybir.InstEventSemaphore(
            name=f"WSPLIT-{ctr[0]}",
            engine=engine,
            ins=[],
            outs=[],
            sync_info=mybir.SyncInfo(on_wait=[wait], on_update=[]),
        )

    for f in nc.m.functions:
        for blk in f.blocks:
            insts = blk.instructions
            out = []
            for inst in insts:
                si = inst.sync_info
                if si is not None and len(si.on_wait) > 1:
                    waits = list(si.on_wait)
                    for w in waits[:-1]:
                        out.append(mk(inst.engine, w))
                    inst.sync_info = mybir.SyncInfo(
                        on_wait=[waits[-1]], on_update=list(si.on_update)
                    )
                out.append(inst)
            insts[:] = out
    return nc


N_CORES = 8
B, T, C = 128, 256, 384
H, DH = 6, 64
BL = B // N_CORES  # batches per core
GB = 2  # batches per projection group
BF16 = mybir.dt.bfloat16
FP32 = mybir.dt.float32
AFT = mybir.ActivationFunctionType
SCALE = DH**-0.5  # 0.125
NEG = -1.0e9


def build_kernel(bl: int = BL) -> bass.Bass:
    nc = bass.Bass()
    xT = nc.dram_tensor("xT", [bl, C, T], BF16, kind="ExternalInput")
    wqt = nc.dram_tensor("wqt", [C, C], BF16, kind="ExternalInput")  # Wq.T [C, D]
    wkt = nc.dram_tensor("wkt", [C, C], BF16, kind="ExternalInput")
    wvt = nc.dram_tensor("wvt", [C, C], BF16, kind="ExternalInput")
    wot = nc.dram_tensor("wot", [C, C], BF16, kind="ExternalInput")  # Wo.T [D, C]
    y = nc.dram_tensor("y", [bl, T, C], FP32, kind="ExternalOutput")

    with tile.TileContext(nc) as tc:
        with (
            tc.tile_pool(name="const", bufs=1) as const,
            tc.tile_pool(name="xp", bufs=2) as xp,
            tc.tile_pool(name="qkp", bufs=2) as qkp,
            tc.tile_pool(name="vp", bufs=3) as vp,
            tc.tile_pool(name="ptp", bufs=4) as ptp,
            tc.tile_pool(name="osp", bufs=2) as osp,
            tc.tile_pool(name="otp", bufs=2) as otp,
            tc.tile_pool(name="rsp", bufs=4) as rsp,
            tc.tile_pool(name="ysbp", bufs=4) as ysbp,
            tc.tile_pool(name="psB", bufs=4, space="PSUM") as psB,
            tc.tile_pool(name="psS", bufs=2, space="PSUM") as psS,
            tc.tile_pool(name="psOT", bufs=1, space="PSUM") as psOT,
            tc.tile_pool(name="psY", bufs=1, space="PSUM") as psY,
        ):
            # ---- constants ----
            # strict lower-triangular (in [ts, tq]: 1 where tq < ts) bf16
            tri = const.tile([128, 128], BF16)
            nc.gpsimd.memset(tri, 1.0)
            nc.gpsimd.affine_select(
                out=tri, in_=tri, compare_op=mybir.AluOpType.is_ge,
                fill=0.0, base=-1, pattern=[[-1, 128]], channel_multiplier=1,
            )
            negI = const.tile([128, 128], BF16)
            make_identity(nc, negI)
            nc.vector.tensor_scalar_mul(negI, negI, NEG)
            ident = const.tile([128, 128], BF16)
            make_identity(nc, ident)

            w_sb = {}
            for name, dram in (("wq", wqt), ("wk", wkt), ("wv", wvt), ("wo", wot)):
                w = const.tile([128, 3, C], BF16, tag=name)
                nc.sync.dma_start(out=w, in_=dram.rearrange("(k p) d -> p k d", p=128))
                w_sb[name] = w

            n_g = bl // GB
            # deferred work queues: batches whose O-transpose / Y-projection
            # have not been emitted yet (emitted interleaved into later
            # batches' attention so the PE never waits on the norm chain)
            pend_tr = []  # (o_sb, ot_tile, b)
            pend_y = []  # (ot, b)

            def emit_tr(o_sb, ot):
                otp_ps = psOT.tile([128, 3, T], BF16, tag="otps", name="otps")
                for dg in range(3):
                    nc.tensor.transpose(
                        otp_ps[:, dg, 0:128],
                        o_sb[:, 0, dg * 128 : (dg + 1) * 128], ident,
                    )
                    nc.tensor.transpose(
                        otp_ps[:, dg, 128:256],
                        o_sb[:, 1, dg * 128 : (dg + 1) * 128], ident,
                    )
                nc.vector.tensor_copy(ot, otp_ps)

            def emit_y_half(ot, b, t2):
                ys = psY.tile([128, C], FP32, tag="ys", name=f"ys{t2}")
                for k in range(3):
                    nc.tensor.matmul(
                        ys,
                        lhsT=ot[:, k, t2 * 128 : (t2 + 1) * 128],
                        rhs=w_sb["wo"][:, k, :],
                        start=(k == 0),
                        stop=(k == 2),
                    )
                ysb = ysbp.tile([128, C], FP32, tag="ysb", name=f"ysb{t2}")
                if t2 == 0:
                    nc.scalar.copy(ysb, ys)
                else:
                    nc.vector.tensor_copy(ysb, ys)
                nc.sync.dma_start(
                    out=y[b, t2 * 128 : (t2 + 1) * 128, :], in_=ysb
                )

            for g in range(n_g):
                # ---- load xT for GB batches: [128, k, b, T] ----
                xt = xp.tile([128, 3, GB, T], BF16)
                for bi in range(GB):
                    nc.sync.dma_start(
                        out=xt[:, :, bi, :],
                        in_=xT[g * GB + bi].rearrange("(k p) t -> p k t", p=128),
                    )

                # ---- QT/KT for both batches: [D, b, T] ----
                qt = qkp.tile([128, 3, GB, T], BF16, tag="qt")
                kt = qkp.tile([128, 3, GB, T], BF16, tag="kt")
                ci = 0
                for dst, wname in ((qt, "wq"), (kt, "wk")):
                    w = w_sb[wname]
                    for d in range(3):
                        ps = psB.tile([128, GB * T], FP32, tag="big")
                        for k in range(3):
                            nc.tensor.matmul(
                                ps,
                                lhsT=w[:, k, d * 128 : (d + 1) * 128],
                                rhs=xt[:, k, :, :],
                                start=(k == 0),
                                stop=(k == 2),
                            )
                        if ci % 2 == 0:
                            nc.scalar.copy(dst[:, d, :, :], ps)
                        else:
                            nc.vector.tensor_copy(dst[:, d, :, :], ps)
                        ci += 1

                # ---- V = [ts, head, 64|ones] per batch ----
                vs = []
                for bi in range(GB):
                    v = vp.tile([128, 2, H, 65], BF16, tag="v")
                    nc.gpsimd.memset(v[:, :, :, 64:65], 1.0)
                    for t2 in range(2):
                        ps = psB.tile([128, GB * T], FP32, tag="big")
                        for k in range(3):
                            nc.tensor.matmul(
                                ps[:, 0:C],
                                lhsT=xt[:, k, bi, t2 * 128 : (t2 + 1) * 128],
                                rhs=w_sb["wv"][:, k, :],
                                start=(k == 0),
                                stop=(k == 2),
                            )
                        nc.vector.tensor_copy(
                            v[:, t2, :, 0:64],
                            ps[:, 0:C].rearrange("p (h d) -> p h d", h=H),
                        )
                    vs.append(v)

                # ---- attention per batch ----
                for bi in range(GB):
                    b = g * GB + bi
                    v = vs[bi]
                    # O accumulators, one per tq-half: [128, head, 64|sum]
                    op0 = psB.tile([128, GB * T], FP32, tag="big", name="op0")
                    op1 = psB.tile([128, GB * T], FP32, tag="big", name="op1")
                    o0 = op0[:, 0 : H * 65].rearrange("p (h d) -> p h d", h=H)
                    o1 = op1[:, 0 : H * 65].rearrange("p (h d) -> p h d", h=H)
                    pt_tiles = {}

                    def emit_scores(p, bi=bi, pt_tiles=pt_tiles, qt=qt, kt=kt):
                        for sub in range(2):
                            h = 2 * p + sub
                            doff = sub * 64
                            qh = qt[doff : doff + 64, p, bi, :]
                            kh = kt[doff : doff + 64, p, bi, :]
                            sc = psS.tile([128, 2, T], FP32, tag="sc",
                                          name=f"sc_{h}")
                            # ts 0:128 x tq 0:128 (diagonal block, masked)
                            nc.tensor.matmul(
                                sc[:, 0, 0:128], lhsT=kh[:, 0:128],
                                rhs=qh[:, 0:128], start=True, stop=False,
                            )
                            nc.tensor.matmul(
                                sc[:, 0, 0:128], lhsT=negI, rhs=tri,
                                start=False, stop=True,
                            )
                            # ts 0:128 x tq 128:256 (full block)
                            nc.tensor.matmul(
                                sc[:, 0, 128:256], lhsT=kh[:, 0:128],
                                rhs=qh[:, 128:256], start=True, stop=True,
                            )
                            # ts 128:256 x tq 128:256 (diagonal block, masked)
                            nc.tensor.matmul(
                                sc[:, 1, 128:256], lhsT=kh[:, 128:256],
                                rhs=qh[:, 128:256], start=True, stop=False,
                            )
                            nc.tensor.matmul(
                                sc[:, 1, 128:256], lhsT=negI, rhs=tri,
                                start=False, stop=True,
                            )
                            pt = ptp.tile([128, 2, T], BF16, tag="pt",
                                          name=f"pt_{h}")
                            nc.scalar.activation(pt[:, 0, :], sc[:, 0, :],
                                                 AFT.Exp, scale=SCALE)
                            nc.scalar.activation(pt[:, 1, 128:256],
                                                 sc[:, 1, 128:256],
                                                 AFT.Exp, scale=SCALE)
                            pt_tiles[h] = pt

                    def emit_av(p, v=v, o0=o0, o1=o1, pt_tiles=pt_tiles):
                        for sub in range(2):
                            h = 2 * p + sub
                            pt = pt_tiles[h]
                            # tq 0:128 sees only ts 0:128
                            nc.tensor.matmul(
                                o0[:, h, :], lhsT=pt[:, 0, 0:128],
                                rhs=v[:, 0, h, :], start=True, stop=True,
                            )
                            # tq 128:256 sees both ts groups
                            nc.tensor.matmul(
                                o1[:, h, :], lhsT=pt[:, 0, 128:256],
                                rhs=v[:, 0, h, :], start=True, stop=False,
                            )
                            nc.tensor.matmul(
                                o1[:, h, :], lhsT=pt[:, 1, 128:256],
                                rhs=v[:, 1, h, :], start=False, stop=True,
                            )

                    # normalized O in SBUF [tq-half, D] bf16
                    o_sb = osp.tile([128, 2, C], BF16, tag="osb")

                    def emit_norm(o_sb=o_sb, o0=o0, o1=o1):
                        for half, op_ in ((0, o0), (1, o1)):
                            rs = rsp.tile([128, H], FP32, tag="rs",
                                          name=f"rs{half}")
                            nc.vector.reciprocal(rs, op_[:, :, 64:65])
                            for h in range(H):
                                dst = o_sb[:, half, h * 64 : (h + 1) * 64]
                                if h % 2 == 0:
                                    nc.scalar.activation(
                                        dst, op_[:, h, 0:64], AFT.Copy,
                                        scale=rs[:, h : h + 1],
                                    )
                                else:
                                    nc.vector.tensor_scalar_mul(
                                        dst, op_[:, h, 0:64], rs[:, h : h + 1]
                                    )

                    emit_scores(0)
                    emit_scores(1)
                    emit_av(0)
                    if pend_y:
                        emit_y_half(*pend_y[0], 0)
                    emit_scores(2)
                    emit_av(1)
                    if pend_y:
                        emit_y_half(*pend_y.pop(0), 1)
                    emit_av(2)
                    emit_norm()
                    if pend_tr:
                        o_prev, ot_prev, b_prev = pend_tr.pop(0)
                        emit_tr(o_prev, ot_prev)
                        pend_y.append((ot_prev, b_prev))
                    ot_t = otp.tile([128, 3, T], BF16, tag="ot")
                    pend_tr.append((o_sb, ot_t, b))

            # drain deferred work
            while pend_tr or pend_y:
                if pend_y:
                    ot_, b_ = pend_y.pop(0)
                    emit_y_half(ot_, b_, 0)
                    emit_y_half(ot_, b_, 1)
                if pend_tr:
                    o_prev, ot_prev, b_prev = pend_tr.pop(0)
                    emit_tr(o_prev, ot_prev)
                    pend_y.append((ot_prev, b_prev))
    return nc


_NC = None


def _get_nc():
    global _NC
    if _NC is None:
        _NC = split_multi_waits(build_kernel())
    return _NC


def kernel(x, Wq, Wk, Wv, Wo, _trace=False):
    bf16 = ml_dtypes.bfloat16
    wq_t = np.ascontiguousarray(Wq.T).astype(bf16)
    wk_t = np.ascontiguousarray(Wk.T).astype(bf16)
    wv_t = np.ascontiguousarray(Wv.T).astype(bf16)
    wo_t = np.ascontiguousarray(Wo.T).astype(bf16)
    in_maps = []
    for i in range(N_CORES):
        xs = x[i * BL : (i + 1) * BL]  # [BL, T, C]
        xs_t = np.ascontiguousarray(xs.transpose(0, 2, 1)).astype(bf16)
        in_maps.append(
            {"xT": xs_t, "wqt": wq_t, "wkt": wk_t, "wvt": wv_t, "wot": wo_t}
        )
    res = run_bass_kernel_spmd(
        _get_nc(), in_maps, list(range(N_CORES)), trace=_trace
    )
    out = np.concatenate([r["y"] for r in res.results], axis=0)
    if _trace:
        return out.astype(np.float32), res
    return out.astype(np.float32)


# revision 4
# speedup vs baseline: 1.9122x; 1.1964x over previous
"""Causal multi-head attention (B=128, T=256, C=384, H=6, Dh=64) on 8 TRN2
NeuronCores, data-parallel over batch (16 batches per core, no collectives).

v2: transposed-scores formulation — P is never transposed.
  - scores computed directly as ST[ts, tq] = KT_h.T @ QT_h (K stationary), so
    exp(ST) = unnormalized P^T feeds the AV matmul with no transpose
  - causal mask applied by accumulating -1e9 * TRI into the score PSUM via a
    (-1e9*I) stationary matmul before the exp (exp -> exact 0)
  - AV uses the P^T blocks as the *stationary* operand and [V_h | ones] as
    moving, so O lands as [tq, d] with the softmax denominator in column 64
    -> normalization is a per-partition tensor_scalar (no cross-partition
    broadcast needed)
  - O is transposed back to OT [d, tq] on the PE (6 bf16 128x128 transposes
    per batch, ~30x less transpose work than transposing P)
  - output projection consumes OT [D, tq] as stationary; PSUM pressure: one
    shared "big" pool (QK/V/O) + scores + OT + Y = 8 banks exactly
"""

import sys

sys.path.insert(0, "/opt/trn_rl_repo")

import numpy as np
import ml_dtypes

import concourse.bass as bass
import concourse.tile as tile
from concourse import mybir
from concourse.bass_utils import run_bass_kernel_spmd
from concourse.masks import make_identity


def split_multi_waits(nc):
    """This walrus build accepts at most one sync-wait command per
    instruction; hoist extra waits into standalone InstEventSemaphore
    instructions on the same engine queue (queue waits run in order before
    the original instruction, so semantics are preserved)."""
    ctr = [0]

    def mk(engine, wait):
        ctr[0] += 1
        return mybir.InstEventSemaphore(
            name=f"WSPLIT-{ctr[0]}",
            engine=engine,
            ins=[],
            outs=[],
            sync_info=mybir.SyncInfo(on_wait=[wait], on_update=[]),
        )

    for f in nc.m.functions:
        for blk in f.blocks:
            insts = blk.instructions
            out = []
            for inst in insts:
                si = inst.sync_info
                if si is not None and len(si.on_wait) > 1:
                    waits = list(si.on_wait)
                    for w in waits[:-1]:
                        out.append(mk(inst.engine, w))
                    inst.sync_info = mybir.SyncInfo(
                        on_wait=[waits[-1]], on_update=list(si.on_update)
                    )
                out.append(inst)
            insts[:] = out
    return nc


N_CORES = 8
B, T, C = 128, 256, 384
H, DH = 6, 64
BL = B // N_CORES  # batches per core
GB = 2  # batches per projection group
BF16 = mybir.dt.bfloat16
FP32 = mybir.dt.float32
AFT = mybir.ActivationFunctionType
SCALE = DH**-0.5  # 0.125
NEG = -1.0e9


def build_kernel(bl: int = BL) -> bass.Bass:
    nc = bass.Bass()
    xT = nc.dram_tensor("xT", [bl, C, T], BF16, kind="ExternalInput")
    wqt = nc.dram_tensor("wqt", [C, C], BF16, kind="ExternalInput")  # Wq.T [C, D]
    wkt = nc.dram_tensor("wkt", [C, C], BF16, kind="ExternalInput")
    wvt = nc.dram_tensor("wvt", [C, C], BF16, kind="ExternalInput")
    wot = nc.dram_tensor("wot", [C, C], BF16, kind="ExternalInput")  # Wo.T [D, C]
    y = nc.dram_tensor("y", [bl, T, C], FP32, kind="ExternalOutput")

    with tile.TileContext(nc) as tc:
        with (
            tc.tile_pool(name="const", bufs=1) as const,
            tc.tile_pool(name="xp", bufs=2) as xp,
            tc.tile_pool(name="qkp", bufs=2) as qkp,
            tc.tile_pool(name="vp", bufs=3) as vp,
            tc.tile_pool(name="ptp", bufs=4) as ptp,
            tc.tile_pool(name="osp", bufs=2) as osp,
            tc.tile_pool(name="otp", bufs=2) as otp,
            tc.tile_pool(name="rsp", bufs=4) as rsp,
            tc.tile_pool(name="ysbp", bufs=4) as ysbp,
            tc.tile_pool(name="psB", bufs=4, space="PSUM") as psB,
            tc.tile_pool(name="psS", bufs=2, space="PSUM") as psS,
            tc.tile_pool(name="psOT", bufs=1, space="PSUM") as psOT,
            tc.tile_pool(name="psY", bufs=1, space="PSUM") as psY,
        ):
            # ---- constants ----
            # strict lower-triangular (in [ts, tq]: 1 where tq < ts) bf16
            tri = const.tile([128, 128], BF16)
            nc.gpsimd.memset(tri, 1.0)
            nc.gpsimd.affine_select(
                out=tri, in_=tri, compare_op=mybir.AluOpType.is_ge,
                fill=0.0, base=-1, pattern=[[-1, 128]], channel_multiplier=1,
            )
            negI = const.tile([128, 128], BF16)
            make_identity(nc, negI)
            nc.vector.tensor_scalar_mul(negI, negI, NEG)
            ident = const.tile([128, 128], BF16)
            make_identity(nc, ident)

            # first batch-group activations DMA'd before the weights: the
            # Sync queue issues serially and the first QK matmul needs x
            xt0 = xp.tile([128, 3, GB, T], BF16, tag="xt", name="xt0")
            for bi in range(GB):
                nc.sync.dma_start(
                    out=xt0[:, :, bi, :],
                    in_=xT[bi].rearrange("(k p) t -> p k t", p=128),
                )
            w_sb = {}
            for name, dram in (("wq", wqt), ("wk", wkt), ("wv", wvt), ("wo", wot)):
                w = const.tile([128, 3, C], BF16, tag=name)
                nc.sync.dma_start(out=w, in_=dram.rearrange("(k p) d -> p k d", p=128))
                w_sb[name] = w

            n_g = bl // GB
            # deferred work queues: batches whose O-transpose / Y-projection
            # have not been emitted yet (emitted interleaved into later
            # batches' attention so the PE never waits on the norm chain)
            pend_tr = []  # (o_sb, ot_tile, b)
            pend_y = []  # (ot, b)

            def emit_tr(o_sb, ot):
                otp_ps = psOT.tile([128, 3, T], BF16, tag="otps", name="otps")
                for dg in range(3):
                    nc.tensor.transpose(
                        otp_ps[:, dg, 0:128],
                        o_sb[:, 0, dg * 128 : (dg + 1) * 128], ident,
                    )
                    nc.tensor.transpose(
                        otp_ps[:, dg, 128:256],
                        o_sb[:, 1, dg * 128 : (dg + 1) * 128], ident,
                    )
                nc.vector.tensor_copy(ot, otp_ps)

            def emit_y_half(ot, b, t2):
                ys = psY.tile([128, C], FP32, tag="ys", name=f"ys{t2}")
                for k in range(3):
                    nc.tensor.matmul(
                        ys,
                        lhsT=ot[:, k, t2 * 128 : (t2 + 1) * 128],
                        rhs=w_sb["wo"][:, k, :],
                        start=(k == 0),
                        stop=(k == 2),
                    )
                ysb = ysbp.tile([128, C], FP32, tag="ysb", name=f"ysb{t2}")
                if t2 == 0:
                    nc.scalar.copy(ysb, ys)
                else:
                    nc.vector.tensor_copy(ysb, ys)
                nc.sync.dma_start(
                    out=y[b, t2 * 128 : (t2 + 1) * 128, :], in_=ysb
                )

            for g in range(n_g):
                # ---- load xT for GB batches: [128, k, b, T] ----
                if g == 0:
                    xt = xt0
                else:
                    xt = xp.tile([128, 3, GB, T], BF16, tag="xt")
                    for bi in range(GB):
                        nc.sync.dma_start(
                            out=xt[:, :, bi, :],
                            in_=xT[g * GB + bi].rearrange(
                                "(k p) t -> p k t", p=128),
                        )

                # ---- QT/KT for both batches: [D, b, T] ----
                qt = qkp.tile([128, 3, GB, T], BF16, tag="qt")
                kt = qkp.tile([128, 3, GB, T], BF16, tag="kt")
                # allocate all 6 PSUM tiles up front but run the groups that
                # reuse the previous batch's O accumulators (alloc index 2,3)
                # last, so the PE never waits on the norm chain; this order
                # also produces q0/k0 first, which the first scores need
                qk_ps = [psB.tile([128, GB * T], FP32, tag="big",
                                  name=f"qkps{i}") for i in range(6)]
                order = [(qt, "wq", 0, 0), (qt, "wq", 1, 1),
                         (kt, "wk", 0, 4), (kt, "wk", 1, 5),
                         (qt, "wq", 2, 2), (kt, "wk", 2, 3)]
                for ci, (dst, wname, d, pi) in enumerate(order):
                    w = w_sb[wname]
                    ps = qk_ps[pi]
                    for k in range(3):
                        nc.tensor.matmul(
                            ps,
                            lhsT=w[:, k, d * 128 : (d + 1) * 128],
                            rhs=xt[:, k, :, :],
                            start=(k == 0),
                            stop=(k == 2),
                        )
                    if ci % 2 == 0:
                        nc.scalar.copy(dst[:, d, :, :], ps)
                    else:
                        nc.vector.tensor_copy(dst[:, d, :, :], ps)

                # ---- V = [ts, head, 64|ones] per batch ----
                vs = []
                for bi in range(GB):
                    v = vp.tile([128, 2, H, 65], BF16, tag="v")
                    nc.gpsimd.memset(v[:, :, :, 64:65], 1.0)
                    for t2 in range(2):
                        ps = psB.tile([128, GB * T], FP32, tag="big")
                        for k in range(3):
                            nc.tensor.matmul(
                                ps[:, 0:C],
                                lhsT=xt[:, k, bi, t2 * 128 : (t2 + 1) * 128],
                                rhs=w_sb["wv"][:, k, :],
                                start=(k == 0),
                                stop=(k == 2),
                            )
                        nc.vector.tensor_copy(
                            v[:, t2, :, 0:64],
                            ps[:, 0:C].rearrange("p (h d) -> p h d", h=H),
                        )
                    vs.append(v)

                # ---- attention per batch ----
                for bi in range(GB):
                    b = g * GB + bi
                    v = vs[bi]
                    # O accumulators, one per tq-half: [128, head, 64|sum]
                    op0 = psB.tile([128, GB * T], FP32, tag="big", name="op0")
                    op1 = psB.tile([128, GB * T], FP32, tag="big", name="op1")
                    o0 = op0[:, 0 : H * 65].rearrange("p (h d) -> p h d", h=H)
                    o1 = op1[:, 0 : H * 65].rearrange("p (h d) -> p h d", h=H)
                    pt_tiles = {}

                    def emit_scores(p, bi=bi, pt_tiles=pt_tiles, qt=qt, kt=kt):
                        for sub in range(2):
                            h = 2 * p + sub
                            doff = sub * 64
                            qh = qt[doff : doff + 64, p, bi, :]
                            kh = kt[doff : doff + 64, p, bi, :]
                            # sc cols: 0:256 = ts-grp0 x tq 0:256,
                            #          256:384 = ts-grp1 x tq 128:256
                            sc = psS.tile([128, 384], FP32, tag="sc",
                                          name=f"sc_{h}")
                            # ts 0:128 x tq 0:128 (diagonal block, masked)
                            nc.tensor.matmul(
                                sc[:, 0:128], lhsT=kh[:, 0:128],
                                rhs=qh[:, 0:128], start=True, stop=False,
                            )
                            nc.tensor.matmul(
                                sc[:, 0:128], lhsT=negI, rhs=tri,
                                start=False, stop=True,
                            )
                            # ts 0:128 x tq 128:256 (full block)
                            nc.tensor.matmul(
                                sc[:, 128:256], lhsT=kh[:, 0:128],
                                rhs=qh[:, 128:256], start=True, stop=True,
                            )
                            # ts 128:256 x tq 128:256 (diagonal block, masked)
                            nc.tensor.matmul(
                                sc[:, 256:384], lhsT=kh[:, 128:256],
                                rhs=qh[:, 128:256], start=True, stop=False,
                            )
                            nc.tensor.matmul(
                                sc[:, 256:384], lhsT=negI, rhs=tri,
                                start=False, stop=True,
                            )
                            pt = ptp.tile([128, 384], BF16, tag="pt",
                                          name=f"pt_{h}")
                            nc.scalar.activation(pt, sc, AFT.Exp, scale=SCALE)
                            pt_tiles[h] = pt

                    def emit_av(p, v=v, o0=o0, o1=o1, pt_tiles=pt_tiles):
                        for sub in range(2):
                            h = 2 * p + sub
                            pt = pt_tiles[h]
                            # tq 0:128 sees only ts 0:128
                            nc.tensor.matmul(
                                o0[:, h, :], lhsT=pt[:, 0:128],
                                rhs=v[:, 0, h, :], start=True, stop=True,
                            )
                            # tq 128:256 sees both ts groups
                            nc.tensor.matmul(
                                o1[:, h, :], lhsT=pt[:, 128:256],
                                rhs=v[:, 0, h, :], start=True, stop=False,
                            )
                            nc.tensor.matmul(
                                o1[:, h, :], lhsT=pt[:, 256:384],
                                rhs=v[:, 1, h, :], start=False, stop=True,
                            )

                    # normalized O in SBUF [tq-half, D] bf16
                    o_sb = osp.tile([128, 2, C], BF16, tag="osb")

                    def emit_norm(o_sb=o_sb, o0=o0, o1=o1):
                        for half, op_ in ((0, o0), (1, o1)):
                            rs = rsp.tile([128, H], FP32, tag="rs",
                                          name=f"rs{half}")
                            nc.vector.reciprocal(rs, op_[:, :, 64:65])
                            for h in range(H):
                                dst = o_sb[:, half, h * 64 : (h + 1) * 64]
                                if h % 2 == 0:
                                    nc.scalar.activation(
                                        dst, op_[:, h, 0:64], AFT.Copy,
                                        scale=rs[:, h : h + 1],
                                    )
                                else:
                                    nc.vector.tensor_scalar_mul(
                                        dst, op_[:, h, 0:64], rs[:, h : h + 1]
                                    )

                    emit_scores(0)
                    emit_scores(1)
                    emit_av(0)
                    if pend_y:
                        emit_y_half(*pend_y[0], 0)
                    emit_scores(2)
                    emit_av(1)
                    if pend_y:
                        emit_y_half(*pend_y.pop(0), 1)
                    emit_av(2)
                    emit_norm()
                    if pend_tr:
                        o_prev, ot_prev, b_prev = pend_tr.pop(0)
                        emit_tr(o_prev, ot_prev)
                        pend_y.append((ot_prev, b_prev))
                    ot_t = otp.tile([128, 3, T], BF16, tag="ot")
                    pend_tr.append((o_sb, ot_t, b))

            # drain deferred work: transpose first so its Y overlaps the
            # earlier batch's Y chain
            while pend_tr:
                o_prev, ot_prev, b_prev = pend_tr.pop(0)
                emit_tr(o_prev, ot_prev)
                pend_y.append((ot_prev, b_prev))
            while pend_y:
                ot_, b_ = pend_y.pop(0)
                emit_y_half(ot_, b_, 0)
                emit_y_half(ot_, b_, 1)
    return nc


_NC = None


def _get_nc():
    global _NC
    if _NC is None:
        _NC = split_multi_waits(build_kernel())
    return _NC


def kernel(x, Wq, Wk, Wv, Wo, _trace=False):
    bf16 = ml_dtypes.bfloat16
    wq_t = np.ascontiguousarray(Wq.T).astype(bf16)
    wk_t = np.ascontiguousarray(Wk.T).astype(bf16)
    wv_t = np.ascontiguousarray(Wv.T).astype(bf16)
    wo_t = np.ascontiguousarray(Wo.T).astype(bf16)
    in_maps = []
    for i in range(N_CORES):
        xs = x[i * BL : (i + 1) * BL]  # [BL, T, C]
        xs_t = np.ascontiguousarray(xs.transpose(0, 2, 1)).astype(bf16)
        in_maps.append(
            {"xT": xs_t, "wqt": wq_t, "wkt": wk_t, "wvt": wv_t, "wot": wo_t}
        )
    res = run_bass_kernel_spmd(
        _get_nc(), in_maps, list(range(N_CORES)), trace=_trace
    )
    out = np.concatenate([r["y"] for r in res.results], axis=0)
    if _trace:
        return out.astype(np.float32), res
    return out.astype(np.float32)


# revision 5
# speedup vs baseline: 1.9490x; 1.0192x over previous
"""Causal multi-head attention (B=128, T=256, C=384, H=6, Dh=64) on 8 TRN2
NeuronCores, data-parallel over batch (16 batches per core, no collectives).

v2: transposed-scores formulation — P is never transposed.
  - scores computed directly as ST[ts, tq] = KT_h.T @ QT_h (K stationary), so
    exp(ST) = unnormalized P^T feeds the AV matmul with no transpose
  - causal mask applied by accumulating -1e9 * TRI into the score PSUM via a
    (-1e9*I) stationary matmul before the exp (exp -> exact 0)
  - AV uses the P^T blocks as the *stationary* operand and [V_h | ones] as
    moving, so O lands as [tq, d] with the softmax denominator in column 64
    -> normalization is a per-partition tensor_scalar (no cross-partition
    broadcast needed)
  - O is transposed back to OT [d, tq] on the PE (6 bf16 128x128 transposes
    per batch, ~30x less transpose work than transposing P)
  - output projection consumes OT [D, tq] as stationary; PSUM pressure: one
    shared "big" pool (QK/V/O) + scores + OT + Y = 8 banks exactly
"""

import sys

sys.path.insert(0, "/opt/trn_rl_repo")

import numpy as np
import ml_dtypes

import concourse.bass as bass
import concourse.tile as tile
from concourse import mybir
from concourse.bass_utils import run_bass_kernel_spmd
from concourse.masks import make_identity


def split_multi_waits(nc):
    """This walrus build accepts at most one sync-wait command per
    instruction; hoist extra waits into standalone InstEventSemaphore
    instructions on the same engine queue (queue waits run in order before
    the original instruction, so semantics are preserved)."""
    ctr = [0]

    def mk(engine, wait):
        ctr[0] += 1
        return mybir.InstEventSemaphore(
            name=f"WSPLIT-{ctr[0]}",
            engine=engine,
            ins=[],
            outs=[],
            sync_info=mybir.SyncInfo(on_wait=[wait], on_update=[]),
        )

    for f in nc.m.functions:
        for blk in f.blocks:
            insts = blk.instructions
            out = []
            for inst in insts:
                si = inst.sync_info
                if si is not None and len(si.on_wait) > 1:
                    waits = list(si.on_wait)
                    for w in waits[:-1]:
                        out.append(mk(inst.engine, w))
                    inst.sync_info = mybir.SyncInfo(
                        on_wait=[waits[-1]], on_update=list(si.on_update)
                    )
                out.append(inst)
            insts[:] = out
    return nc


N_CORES = 8
B, T, C = 128, 256, 384
H, DH = 6, 64
BL = B // N_CORES  # batches per core
GB = 2  # batches per projection group
BF16 = mybir.dt.bfloat16
FP32 = mybir.dt.float32
AFT = mybir.ActivationFunctionType
SCALE = DH**-0.5  # 0.125


def build_kernel(bl: int = BL) -> bass.Bass:
    nc = bass.Bass()
    xT = nc.dram_tensor("xT", [bl, C, T], BF16, kind="ExternalInput")
    wqt = nc.dram_tensor("wqt", [C, C], BF16, kind="ExternalInput")  # Wq.T [C, D]
    wkt = nc.dram_tensor("wkt", [C, C], BF16, kind="ExternalInput")
    wvt = nc.dram_tensor("wvt", [C, C], BF16, kind="ExternalInput")
    wot = nc.dram_tensor("wot", [C, C], BF16, kind="ExternalInput")  # Wo.T [D, C]
    y = nc.dram_tensor("y", [bl, T, C], FP32, kind="ExternalOutput")

    with tile.TileContext(nc) as tc:
        with (
            tc.tile_pool(name="const", bufs=1) as const,
            tc.tile_pool(name="xp", bufs=2) as xp,
            tc.tile_pool(name="qkp", bufs=2) as qkp,
            tc.tile_pool(name="vp", bufs=3) as vp,
            tc.tile_pool(name="ptp", bufs=4) as ptp,
            tc.tile_pool(name="osp", bufs=2) as osp,
            tc.tile_pool(name="otp", bufs=2) as otp,
            tc.tile_pool(name="rsp", bufs=4) as rsp,
            tc.tile_pool(name="ysbp", bufs=4) as ysbp,
            tc.tile_pool(name="psB", bufs=4, space="PSUM") as psB,
            tc.tile_pool(name="psS", bufs=2, space="PSUM") as psS,
            tc.tile_pool(name="psOT", bufs=1, space="PSUM") as psOT,
            tc.tile_pool(name="psY", bufs=1, space="PSUM") as psY,
        ):
            # ---- constants ----
            # causal keep-mask in [ts, tq]: 1 where tq >= ts, else 0 (bf16)
            trik = const.tile([128, 128], BF16)
            nc.gpsimd.memset(trik, 1.0)
            nc.gpsimd.affine_select(
                out=trik, in_=trik, compare_op=mybir.AluOpType.is_ge,
                fill=0.0, base=0, pattern=[[1, 128]], channel_multiplier=-1,
            )
            ident = const.tile([128, 128], BF16)
            make_identity(nc, ident)

            # first batch-group activations DMA'd before the weights: the
            # Sync queue issues serially and the first QK matmul needs x
            xt0 = xp.tile([128, 3, GB, T], BF16, tag="xt", name="xt0")
            for bi in range(GB):
                nc.sync.dma_start(
                    out=xt0[:, :, bi, :],
                    in_=xT[bi].rearrange("(k p) t -> p k t", p=128),
                )
            w_sb = {}
            for name, dram in (("wq", wqt), ("wk", wkt), ("wv", wvt), ("wo", wot)):
                w = const.tile([128, 3, C], BF16, tag=name)
                nc.sync.dma_start(out=w, in_=dram.rearrange("(k p) d -> p k d", p=128))
                w_sb[name] = w

            n_g = bl // GB
            # deferred work queues: batches whose O-transpose / Y-projection
            # have not been emitted yet (emitted interleaved into later
            # batches' attention so the PE never waits on the norm chain)
            pend_tr = []  # (o_sb, ot_tile, b)
            pend_y = []  # (ot, b)

            def emit_tr(o_sb, ot):
                otp_ps = psOT.tile([128, 3, T], BF16, tag="otps", name="otps")
                for dg in range(3):
                    nc.tensor.transpose(
                        otp_ps[:, dg, 0:128],
                        o_sb[:, 0, dg * 128 : (dg + 1) * 128], ident,
                    )
                    nc.tensor.transpose(
                        otp_ps[:, dg, 128:256],
                        o_sb[:, 1, dg * 128 : (dg + 1) * 128], ident,
                    )
                nc.vector.tensor_copy(ot, otp_ps)

            def emit_y_half(ot, b, t2, alt_pool=None):
                pool = alt_pool if alt_pool is not None else psY
                ys = pool.tile([128, C], FP32,
                               tag="sc" if alt_pool is not None else "ys",
                               name=f"ys{t2}")
                for k in range(3):
                    nc.tensor.matmul(
                        ys,
                        lhsT=ot[:, k, t2 * 128 : (t2 + 1) * 128],
                        rhs=w_sb["wo"][:, k, :],
                        start=(k == 0),
                        stop=(k == 2),
                    )
                ysb = ysbp.tile([128, C], FP32, tag="ysb", name=f"ysb{t2}")
                if t2 == 0:
                    nc.scalar.copy(ysb, ys)
                else:
                    nc.vector.tensor_copy(ysb, ys)
                nc.sync.dma_start(
                    out=y[b, t2 * 128 : (t2 + 1) * 128, :], in_=ysb
                )

            for g in range(n_g):
                # ---- load xT for GB batches: [128, k, b, T] ----
                if g == 0:
                    xt = xt0
                else:
                    xt = xp.tile([128, 3, GB, T], BF16, tag="xt")
                    for bi in range(GB):
                        nc.sync.dma_start(
                            out=xt[:, :, bi, :],
                            in_=xT[g * GB + bi].rearrange(
                                "(k p) t -> p k t", p=128),
                        )

                # ---- QT/KT for both batches: [D, b, T] ----
                qt = qkp.tile([128, 3, GB, T], BF16, tag="qt")
                kt = qkp.tile([128, 3, GB, T], BF16, tag="kt")
                # allocate all 6 PSUM tiles up front but run the groups that
                # reuse the previous batch's O accumulators (alloc index 2,3)
                # last, so the PE never waits on the norm chain; this order
                # also produces q0/k0 first, which the first scores need
                qk_ps = [psB.tile([128, GB * T], FP32, tag="big",
                                  name=f"qkps{i}") for i in range(6)]
                order = [(qt, "wq", 0, 0), (qt, "wq", 1, 1),
                         (kt, "wk", 0, 4), (kt, "wk", 1, 5),
                         (qt, "wq", 2, 2), (kt, "wk", 2, 3)]
                for ci, (dst, wname, d, pi) in enumerate(order):
                    w = w_sb[wname]
                    ps = qk_ps[pi]
                    for k in range(3):
                        nc.tensor.matmul(
                            ps,
                            lhsT=w[:, k, d * 128 : (d + 1) * 128],
                            rhs=xt[:, k, :, :],
                            start=(k == 0),
                            stop=(k == 2),
                        )
                    if ci % 2 == 0:
                        nc.scalar.copy(dst[:, d, :, :], ps)
                    else:
                        nc.vector.tensor_copy(dst[:, d, :, :], ps)

                # ---- V = [ts, head, 64|ones] per batch ----
                vs = []
                for bi in range(GB):
                    v = vp.tile([128, 2, H, 65], BF16, tag="v")
                    nc.gpsimd.memset(v[:, :, :, 64:65], 1.0)
                    for t2 in range(2):
                        ps = psB.tile([128, GB * T], FP32, tag="big")
                        for k in range(3):
                            nc.tensor.matmul(
                                ps[:, 0:C],
                                lhsT=xt[:, k, bi, t2 * 128 : (t2 + 1) * 128],
                                rhs=w_sb["wv"][:, k, :],
                                start=(k == 0),
                                stop=(k == 2),
                            )
                        nc.vector.tensor_copy(
                            v[:, t2, :, 0:64],
                            ps[:, 0:C].rearrange("p (h d) -> p h d", h=H),
                        )
                    vs.append(v)

                # ---- attention per batch ----
                for bi in range(GB):
                    b = g * GB + bi
                    v = vs[bi]
                    # O accumulators, one per tq-half: [128, head, 64|sum]
                    op0 = psB.tile([128, GB * T], FP32, tag="big", name="op0")
                    op1 = psB.tile([128, GB * T], FP32, tag="big", name="op1")
                    o0 = op0[:, 0 : H * 65].rearrange("p (h d) -> p h d", h=H)
                    o1 = op1[:, 0 : H * 65].rearrange("p (h d) -> p h d", h=H)
                    pt_tiles = {}

                    def emit_scores(p, bi=bi, pt_tiles=pt_tiles, qt=qt, kt=kt):
                        for sub in range(2):
                            h = 2 * p + sub
                            doff = sub * 64
                            qh = qt[doff : doff + 64, p, bi, :]
                            kh = kt[doff : doff + 64, p, bi, :]
                            # sc cols: 0:256 = ts-grp0 x tq 0:256,
                            #          256:384 = ts-grp1 x tq 128:256
                            sc = psS.tile([128, 384], FP32, tag="sc",
                                          name=f"sc_{h}")
                            nc.tensor.matmul(
                                sc[:, 0:256], lhsT=kh[:, 0:128],
                                rhs=qh[:, 0:256], start=True, stop=True,
                            )
                            nc.tensor.matmul(
                                sc[:, 256:384], lhsT=kh[:, 128:256],
                                rhs=qh[:, 128:256], start=True, stop=True,
                            )
                            pt = ptp.tile([128, 384], BF16, tag="pt",
                                          name=f"pt_{h}")
                            nc.scalar.activation(pt, sc, AFT.Exp, scale=SCALE)
                            # zero the causally-masked diagonal blocks on the
                            # (otherwise idle) Pool engine
                            nc.gpsimd.tensor_mul(pt[:, 0:128],
                                                 pt[:, 0:128], trik)
                            nc.gpsimd.tensor_mul(pt[:, 256:384],
                                                 pt[:, 256:384], trik)
                            pt_tiles[h] = pt

                    def emit_av(p, v=v, o0=o0, o1=o1, pt_tiles=pt_tiles):
                        for sub in range(2):
                            h = 2 * p + sub
                            pt = pt_tiles[h]
                            # tq 0:128 sees only ts 0:128
                            nc.tensor.matmul(
                                o0[:, h, :], lhsT=pt[:, 0:128],
                                rhs=v[:, 0, h, :], start=True, stop=True,
                            )
                            # tq 128:256 sees both ts groups
                            nc.tensor.matmul(
                                o1[:, h, :], lhsT=pt[:, 128:256],
                                rhs=v[:, 0, h, :], start=True, stop=False,
                            )
                            nc.tensor.matmul(
                                o1[:, h, :], lhsT=pt[:, 256:384],
                                rhs=v[:, 1, h, :], start=False, stop=True,
                            )

                    # normalized O in SBUF [tq-half, D] bf16
                    o_sb = osp.tile([128, 2, C], BF16, tag="osb")

                    def emit_norm(o_sb=o_sb, o0=o0, o1=o1):
                        for half, op_ in ((0, o0), (1, o1)):
                            rs = rsp.tile([128, H], FP32, tag="rs",
                                          name=f"rs{half}")
                            nc.vector.reciprocal(rs, op_[:, :, 64:65])
                            for h in range(H):
                                dst = o_sb[:, half, h * 64 : (h + 1) * 64]
                                if h % 3 == 0:
                                    nc.scalar.activation(
                                        dst, op_[:, h, 0:64], AFT.Copy,
                                        scale=rs[:, h : h + 1],
                                    )
                                else:
                                    nc.vector.tensor_scalar_mul(
                                        dst, op_[:, h, 0:64], rs[:, h : h + 1]
                                    )

                    emit_scores(0)
                    emit_scores(1)
                    emit_av(0)
                    if pend_y:
                        emit_y_half(*pend_y[0], 0)
                    emit_scores(2)
                    emit_av(1)
                    if pend_y:
                        emit_y_half(*pend_y.pop(0), 1)
                    emit_av(2)
                    emit_norm()
                    if pend_tr:
                        o_prev, ot_prev, b_prev = pend_tr.pop(0)
                        emit_tr(o_prev, ot_prev)
                        pend_y.append((ot_prev, b_prev))
                    ot_t = otp.tile([128, 3, T], BF16, tag="ot")
                    pend_tr.append((o_sb, ot_t, b))

            # drain deferred work: transpose first so its Y overlaps the
            # earlier batch's Y chain
            while pend_tr:
                o_prev, ot_prev, b_prev = pend_tr.pop(0)
                emit_tr(o_prev, ot_prev)
                pend_y.append((ot_prev, b_prev))
            di = 0
            while pend_y:
                ot_, b_ = pend_y.pop(0)
                emit_y_half(ot_, b_, 0, psS if di % 2 == 0 else None)
                emit_y_half(ot_, b_, 1, psS if di % 2 == 1 else None)
                di += 1
    return nc


_NC = None


def _get_nc():
    global _NC
    if _NC is None:
        _NC = split_multi_waits(build_kernel())
    return _NC


def kernel(x, Wq, Wk, Wv, Wo, _trace=False):
    bf16 = ml_dtypes.bfloat16
    wq_t = np.ascontiguousarray(Wq.T).astype(bf16)
    wk_t = np.ascontiguousarray(Wk.T).astype(bf16)
    wv_t = np.ascontiguousarray(Wv.T).astype(bf16)
    wo_t = np.ascontiguousarray(Wo.T).astype(bf16)
    in_maps = []
    for i in range(N_CORES):
        xs = x[i * BL : (i + 1) * BL]  # [BL, T, C]
        xs_t = np.ascontiguousarray(xs.transpose(0, 2, 1)).astype(bf16)
        in_maps.append(
            {"xT": xs_t, "wqt": wq_t, "wkt": wk_t, "wvt": wv_t, "wot": wo_t}
        )
    res = run_bass_kernel_spmd(
        _get_nc(), in_maps, list(range(N_CORES)), trace=_trace
    )
    out = np.concatenate([r["y"] for r in res.results], axis=0)
    if _trace:
        return out.astype(np.float32), res
    return out.astype(np.float32)
